# revision 38
# baseline (speedup 1.0000x reference)
"""Trainium2 kernel for nn_CodecModel (scatter_memory).

Pipeline (8 NeuronCores, SPMD, full inputs in / full output out):
  Host (sharding only, no model math):
    - f_c/n_c centers from zeta_f/zeta_t (cheap elementwise, used only to BIN
      occurrences); each occurrence is routed to the (core=row-range-of-128,
      window=col-range-of-64) pairs its deposit footprint can touch
      (conservative bounds from global min/max of dF/dT). Duplicates are
      exact-split on device by construction (tents only match in-range rows,
      windows only cover their own columns).
  Device (per core, identical static program):
    - argmax over K=64 via row-max + is_equal one-hot
    - pattern gather via PE matmul with the transposed one-hot as stationary;
      f_c/n_c ride along as extra contraction rows so psum directly yields
      f_hat/n_hat [128,9] plus gathered P*sigmoid(gate)
    - bilinear scatter-add: for each 128-occurrence tile and channel j,
      stationary = row-tent [pt,128rows], moving = col-tent*val [pt,128
      (re|im)], accumulated over a whole column window in PSUM, spilled to an
      SBUF-resident [128,4096] re/im grid slice.
  Output: concatenate per-core row slices; complex64.
"""

import os
import sys

import numpy as np

sys.path.insert(0, "/opt/trn_rl_repo")

import concourse.bass as bass  # noqa: E402
import concourse.tile as tile  # noqa: E402
from concourse import mybir  # noqa: E402

F_DIM, N_DIM, K_PAT, NCH = 1024, 4096, 64, 9
TWO_PI = 2.0 * np.pi
NCORES = 8
ROWS = F_DIM // NCORES  # 128 rows per core
NWIN = 64  # column windows per core
WCOLS = N_DIM // NWIN  # 64 cols per window
FP32 = mybir.dt.float32
BF16 = mybir.dt.bfloat16
I32 = mybir.dt.int32

_DEBUG_SIM = os.environ.get("CODEC_KERNEL_SIM", "0") == "1"
LAST_EXEC_NS = None
LAST_PROFILE = None


def _host_shard(tau, alpha, zeta_f, zeta_t, log_rho, theta, dF, dT):
    """Bin occurrences into (core, window) segments. Returns per-core packed
    alpha and per-occurrence vectors plus the global segment size G_seg."""
    M = alpha.shape[0]
    f_c = (np.float32(F_DIM / TWO_PI) * zeta_f.astype(np.float32)).astype(np.float32)
    n_c = (np.float32(N_DIM / TWO_PI) * zeta_t.astype(np.float32)).astype(np.float32)

    f64c = f_c.astype(np.float64)
    n64c = n_c.astype(np.float64)
    dFmin, dFmax = float(dF.min()), float(dF.max())
    dTmin, dTmax = float(dT.min()), float(dT.max())
    # conservative deposit row/col spans (corner cells), +-1 slack
    row_lo = np.floor(f64c + dFmin).astype(np.int64) - 1
    row_hi = np.floor(f64c + dFmax).astype(np.int64) + 2
    col_lo = np.floor(n64c + dTmin).astype(np.int64) - 1
    col_hi = np.floor(n64c + dTmax).astype(np.int64) + 2
    np.clip(row_lo, 0, F_DIM - 1, out=row_lo)
    np.clip(row_hi, 0, F_DIM - 1, out=row_hi)
    np.clip(col_lo, 0, N_DIM - 1, out=col_lo)
    np.clip(col_hi, 0, N_DIM - 1, out=col_hi)
    c_lo, c_hi = row_lo // ROWS, row_hi // ROWS
    w_lo, w_hi = col_lo // WCOLS, col_hi // WCOLS

    occs, keys = [], []
    for dc in (0, 1):
        for dw in (0, 1):
            m = (c_lo + dc <= c_hi) & (w_lo + dw <= w_hi)
            idx = np.nonzero(m)[0]
            occs.append(idx)
            keys.append((c_lo[idx] + dc) * NWIN + (w_lo[idx] + dw))
    occs = np.concatenate(occs)
    keys = np.concatenate(keys).astype(np.int32)
    order = np.argsort(keys, kind="stable")
    occs, keys = occs[order], keys[order]

    counts = np.bincount(keys, minlength=NCORES * NWIN)
    g_seg = int(max(1, -(-counts.max() // 128)))
    if g_seg > 2:
        g_seg = -(-g_seg // 4) * 4  # round up for NEFF-cache stability
    cap = g_seg * 128
    sel = np.full((NCORES * NWIN, cap), -1, dtype=np.int64)
    starts = np.concatenate(([0], np.cumsum(counts)[:-1]))
    within = np.arange(len(occs)) - starts[keys]
    sel[keys, within] = occs
    sel = sel.reshape(NCORES, NWIN * cap)

    alphas, vecs = [], []
    win_of = np.repeat(np.arange(NWIN, dtype=np.float32) * WCOLS, cap)
    for c in range(NCORES):
        s = sel[c]
        valid = s >= 0
        sc = np.maximum(s, 0)
        a = alpha[sc]  # [L, 64]
        v = np.empty((s.shape[0], 4), dtype=np.float32)
        v[:, 0] = np.where(valid, f_c[sc] - np.float32(c * ROWS), np.float32(-512.0))
        v[:, 1] = np.where(valid, n_c[sc] - win_of, np.float32(0.0))
        v[:, 2] = np.where(valid, log_rho[sc], np.float32(0.0))
        v[:, 3] = np.where(valid, theta[sc], np.float32(0.0))
        alphas.append(np.ascontiguousarray(a))
        vecs.append(v)
    return alphas, vecs, g_seg


def _build_program(g_seg, tau_neg, chunk=8):
    """Build the SPMD Bass program for one core (identical across cores)."""
    from concourse import bacc
    nc = bacc.Bacc("TRN2", target_bir_lowering=False, debug=False)
    L = NWIN * g_seg * 128
    a_in = nc.dram_tensor("alpha_p", [L, K_PAT], FP32, kind="ExternalInput").ap()
    v_in = nc.dram_tensor("vec_p", [L, 4], FP32, kind="ExternalInput").ap()
    braw_in = nc.dram_tensor("bank_raw", [66, 48], FP32, kind="ExternalInput").ap()
    glog_in = nc.dram_tensor("gate_log", [66, 48], FP32, kind="ExternalInput").ap()
    out_re = nc.dram_tensor("out_re", [ROWS, N_DIM], FP32, kind="ExternalOutput").ap()
    out_im = nc.dram_tensor("out_im", [ROWS, N_DIM], FP32, kind="ExternalOutput").ap()

    ntiles = NWIN * g_seg
    assert ntiles % chunk == 0 or chunk == 1

    with tile.TileContext(nc) as tc:
        _emit(tc, nc, g_seg, tau_neg, chunk,
              a_in, v_in, braw_in, glog_in, out_re, out_im)
    nc.compile()
    return nc


def _emit(tc, nc, g_seg, tau_neg, chunk,
          a_in, v_in, braw_in, glog_in, out_re, out_im):
    from contextlib import ExitStack
    ctx = ExitStack()
    with ctx:
        cpool = ctx.enter_context(tc.tile_pool(name="const", bufs=1))
        gpool = ctx.enter_context(tc.tile_pool(name="grid", bufs=1))
        apool = ctx.enter_context(tc.tile_pool(name="alpha", bufs=3))
        vpool = ctx.enter_context(tc.tile_pool(name="vec", bufs=3))
        wpool = ctx.enter_context(tc.tile_pool(name="work", bufs=3))
        dpool = ctx.enter_context(tc.tile_pool(name="dep", bufs=3))
        spool = ctx.enter_context(tc.tile_pool(name="stat", bufs=3))
        pwin = ctx.enter_context(tc.tile_pool(name="pwin", bufs=2, space="PSUM"))
        pocc = ctx.enter_context(tc.tile_pool(name="pocc", bufs=2, space="PSUM"))
        ptr = ctx.enter_context(tc.tile_pool(name="ptr", bufs=2, space="PSUM"))

        # ---- constants ----
        iota_r_i = cpool.tile([128, 128], FP32)
        nc.gpsimd.iota(iota_r_i[:], pattern=[[1, 128]], base=0, channel_multiplier=0,
                       allow_small_or_imprecise_dtypes=True)
        iota_p_i = cpool.tile([128, 1], FP32)
        nc.gpsimd.iota(iota_p_i[:], pattern=[[0, 1]], base=0, channel_multiplier=1,
                       allow_small_or_imprecise_dtypes=True)
        ident = cpool.tile([128, 128], FP32)
        nc.vector.tensor_scalar(ident[:], iota_r_i[:], iota_p_i[:], None,
                                mybir.AluOpType.is_equal)
        # r-major row iota [p, r*9+j] = r, bf16; c-major col iota [p, c*9+j] = c
        iota_rmaj = cpool.tile([128, 128 * NCH], BF16)
        nc.gpsimd.iota(iota_rmaj[:], pattern=[[1, 128], [0, NCH]], base=0,
                       channel_multiplier=0, allow_small_or_imprecise_dtypes=True)
        iota_cmaj = cpool.tile([128, WCOLS * NCH], BF16)
        nc.gpsimd.iota(iota_cmaj[:], pattern=[[1, WCOLS], [0, NCH]], base=0,
                       channel_multiplier=0, allow_small_or_imprecise_dtypes=True)

        # ---- pattern bank: bank = bank_raw * sigmoid(gate_logits) ----
        braw = cpool.tile([66, 48], FP32)
        nc.sync.dma_start(braw[:], braw_in[:, :])
        glog = cpool.tile([66, 48], FP32)
        nc.sync.dma_start(glog[:], glog_in[:, :])
        gsig = cpool.tile([66, 48], FP32)
        nc.scalar.activation(gsig[:], glog[:], mybir.ActivationFunctionType.Sigmoid)
        braw_s = cpool.tile([66, 48], FP32)
        nc.scalar.copy(braw_s[:], braw[:])  # funnel deps through ACT clock
        bank = cpool.tile([66, 48], FP32)
        nc.vector.tensor_tensor(bank[:], braw_s[:], gsig[:], mybir.AluOpType.mult)

        negpi = cpool.tile([128, 1], FP32)
        nc.vector.memset(negpi[:], float(-np.pi))
        negone = cpool.tile([128, 1], FP32)
        nc.vector.memset(negone[:], -1.0)
        one = cpool.tile([128, 1], FP32)
        nc.vector.memset(one[:], 1.0)

        # ---- grid ----
        grid_re = gpool.tile([128, N_DIM], FP32)
        grid_im = gpool.tile([128, N_DIM], FP32)
        nc.vector.memset(grid_re[:], 0.0)
        nc.vector.memset(grid_im[:], 0.0)

        red_op = mybir.AluOpType.min if tau_neg else mybir.AluOpType.max

        seg_rows = g_seg * 128
        with tc.For_i(0, NWIN, 1,
                      hint_engines=(mybir.EngineType.DVE,
                                    mybir.EngineType.Activation,
                                    mybir.EngineType.PE,
                                    mybir.EngineType.SP)) as s:
            a_chunk = apool.tile([128, g_seg * K_PAT], FP32, tag="ach")
            src = a_in[bass.ds(s * seg_rows, seg_rows), :].rearrange(
                "(c p) k -> p c k", p=128)
            nc.sync.dma_start(
                a_chunk[:].rearrange("p (c k) -> p c k", k=K_PAT), src)
            v_chunk = vpool.tile([128, g_seg * 4], FP32, tag="vch")
            vsrc = v_in[bass.ds(s * seg_rows, seg_rows), :].rearrange(
                "(c p) k -> p c k", p=128)
            nc.sync.dma_start(
                v_chunk[:].rearrange("p (c k) -> p c k", k=4), vsrc)

            psw = pwin.tile([128, 128], FP32)
            for g in range(g_seg):
                ci = g
                at = a_chunk[:, ci * K_PAT:(ci + 1) * K_PAT]
                fcol = v_chunk[:, ci * 4 + 0:ci * 4 + 1]
                ncol = v_chunk[:, ci * 4 + 1:ci * 4 + 2]
                rcol = v_chunk[:, ci * 4 + 2:ci * 4 + 3]
                tcol = v_chunk[:, ci * 4 + 3:ci * 4 + 4]

                # argmax one-hot (+centers) and transpose
                mx = wpool.tile([128, 1], FP32, tag="mx")
                nc.vector.tensor_reduce(mx[:], at, mybir.AxisListType.X, red_op)
                trin = wpool.tile([128, 66], FP32, tag="trin")
                nc.vector.tensor_scalar(trin[:, 0:64], at, mx[:], None,
                                        mybir.AluOpType.is_equal)
                nc.vector.tensor_copy(trin[:, 64:65], fcol)
                nc.vector.tensor_copy(trin[:, 65:66], ncol)
                pst = ptr.tile([66, 128], FP32, tag="pst")
                nc.tensor.transpose(pst[:], trin[:], ident[:])
                statg = spool.tile([66, 128], FP32, tag="statg")
                nc.scalar.copy(statg[:], pst[:])

                # gather matmul -> [128 occ, 48]: fhat|nhat|PGre|PGim
                pso = pocc.tile([128, 48], FP32, tag="pso")
                nc.tensor.matmul(pso[:], statg[:], bank[:], start=True, stop=True)
                occ = wpool.tile([128, 48], FP32, tag="occ")
                nc.vector.tensor_copy(occ[:], pso[:])

                # amplitude
                rho = wpool.tile([128, 1], FP32, tag="rho")
                nc.scalar.activation(rho[:], rcol, mybir.ActivationFunctionType.Exp)
                # range-reduce: sin(x-pi) with x=(theta+c) mod 2pi
                thc = wpool.tile([128, 1], FP32, tag="thc")
                nc.vector.tensor_scalar(thc[:], tcol, float(1.5 * np.pi), None,
                                        mybir.AluOpType.add)
                gc = wpool.tile([128, 1], FP32, tag="gc")
                nc.vector.tensor_scalar(gc[:], thc[:], float(TWO_PI), None,
                                        mybir.AluOpType.is_ge)
                nc.vector.scalar_tensor_tensor(thc[:], gc[:], float(-TWO_PI), thc[:],
                                               mybir.AluOpType.mult,
                                               mybir.AluOpType.add)
                cth = wpool.tile([128, 1], FP32, tag="cth")
                nc.scalar.activation(cth[:], thc[:], mybir.ActivationFunctionType.Sin,
                                     bias=negpi[:])
                ths = wpool.tile([128, 1], FP32, tag="ths")
                nc.vector.tensor_scalar(ths[:], tcol, float(np.pi), None,
                                        mybir.AluOpType.add)
                gs = wpool.tile([128, 1], FP32, tag="gs")
                nc.vector.tensor_scalar(gs[:], ths[:], float(TWO_PI), None,
                                        mybir.AluOpType.is_ge)
                nc.vector.scalar_tensor_tensor(ths[:], gs[:], float(-TWO_PI), ths[:],
                                               mybir.AluOpType.mult,
                                               mybir.AluOpType.add)
                sth = wpool.tile([128, 1], FP32, tag="sth")
                nc.scalar.activation(sth[:], ths[:], mybir.ActivationFunctionType.Sin,
                                     bias=negpi[:])
                are = wpool.tile([128, 1], FP32, tag="are")
                nc.vector.tensor_tensor(are[:], rho[:], cth[:], mybir.AluOpType.mult)
                aim = wpool.tile([128, 1], FP32, tag="aim")
                nc.vector.tensor_tensor(aim[:], rho[:], sth[:], mybir.AluOpType.mult)
                aimn = wpool.tile([128, 1], FP32, tag="aimn")
                nc.vector.tensor_scalar(aimn[:], aim[:], -1.0, None,
                                        mybir.AluOpType.mult)

                # staging: [0:18]=f0|t0  [18:36]=wf|wt  [36:45]=vre  [45:54]=vim
                sf = wpool.tile([128, 54], FP32, tag="sf")
                MAGIC = 8388608.0  # 2**23: x+MAGIC-MAGIC rounds to nearest int
                rnd = wpool.tile([128, 18], FP32, tag="rnd")
                nc.vector.tensor_scalar(rnd[:], occ[:, 0:18], MAGIC, MAGIC,
                                        mybir.AluOpType.add,
                                        mybir.AluOpType.subtract)
                ggt = wpool.tile([128, 18], FP32, tag="ggt")
                nc.vector.tensor_tensor(ggt[:], rnd[:], occ[:, 0:18],
                                        mybir.AluOpType.is_gt)
                nc.vector.tensor_tensor(sf[:, 0:18], rnd[:], ggt[:],
                                        mybir.AluOpType.subtract)
                nc.vector.tensor_tensor(sf[:, 18:36], occ[:, 0:18], sf[:, 0:18],
                                        mybir.AluOpType.subtract)
                t1 = wpool.tile([128, 9], FP32, tag="t1")
                nc.vector.tensor_scalar(t1[:], occ[:, 18:27], are[:], None,
                                        mybir.AluOpType.mult)
                nc.vector.scalar_tensor_tensor(sf[:, 36:45], occ[:, 27:36], aimn[:],
                                               t1[:], mybir.AluOpType.mult,
                                               mybir.AluOpType.add)
                t2 = wpool.tile([128, 9], FP32, tag="t2")
                nc.vector.tensor_scalar(t2[:], occ[:, 18:27], aim[:], None,
                                        mybir.AluOpType.mult)
                nc.vector.scalar_tensor_tensor(sf[:, 45:54], occ[:, 27:36], are[:],
                                               t2[:], mybir.AluOpType.mult,
                                               mybir.AluOpType.add)
                stg = wpool.tile([128, 54], BF16, tag="stg")
                nc.vector.tensor_copy(stg[:], sf[:])

                # row tents [p, r*9+j] bf16
                drow = dpool.tile([128, 128 * NCH], BF16, tag="drow")
                d3 = drow[:].rearrange("p (r j) -> p r j", j=NCH)
                nc.vector.tensor_tensor(
                    d3, iota_rmaj[:].rearrange("p (r j) -> p r j", j=NCH),
                    stg[:, 0:9].unsqueeze(1).broadcast_to((128, 128, NCH)),
                    mybir.AluOpType.subtract)
                nc.vector.tensor_tensor(
                    d3, d3, stg[:, 18:27].unsqueeze(1).broadcast_to((128, 128, NCH)),
                    mybir.AluOpType.subtract)
                rowt = dpool.tile([128, 128 * NCH], BF16, tag="rowt")
                nc.scalar.activation(rowt[:], drow[:],
                                     mybir.ActivationFunctionType.Abs)
                nc.scalar.activation(rowt[:], rowt[:],
                                     mybir.ActivationFunctionType.Relu,
                                     scale=negone[:], bias=one[:])

                # col tents * val -> moving [p, (re|im) c*9+j] bf16
                dcol = dpool.tile([128, WCOLS * NCH], BF16, tag="dcol")
                c3 = dcol[:].rearrange("p (c j) -> p c j", j=NCH)
                nc.vector.tensor_tensor(
                    c3, iota_cmaj[:].rearrange("p (c j) -> p c j", j=NCH),
                    stg[:, 9:18].unsqueeze(1).broadcast_to((128, WCOLS, NCH)),
                    mybir.AluOpType.subtract)
                nc.vector.tensor_tensor(
                    c3, c3, stg[:, 27:36].unsqueeze(1).broadcast_to((128, WCOLS, NCH)),
                    mybir.AluOpType.subtract)
                ca = dpool.tile([128, WCOLS * NCH], BF16, tag="ca")
                nc.vector.tensor_scalar(ca[:], dcol[:], -1.0, 1.0,
                                        mybir.AluOpType.mult,
                                        mybir.AluOpType.add)
                nc.vector.scalar_tensor_tensor(dcol[:], dcol[:], 1.0, ca[:],
                                               mybir.AluOpType.add,
                                               mybir.AluOpType.min)
                nc.vector.tensor_scalar(dcol[:], dcol[:], 0.0, None,
                                        mybir.AluOpType.max)
                xrei = dpool.tile([128, 2 * WCOLS * NCH], BF16, tag="xrei")
                x4 = xrei[:].rearrange("p (h c j) -> p h c j", h=2, j=NCH)
                nc.vector.tensor_tensor(
                    x4[:, 0], c3,
                    stg[:, 36:45].unsqueeze(1).broadcast_to((128, WCOLS, NCH)),
                    mybir.AluOpType.mult)
                nc.vector.tensor_tensor(
                    x4[:, 1], c3,
                    stg[:, 45:54].unsqueeze(1).broadcast_to((128, WCOLS, NCH)),
                    mybir.AluOpType.mult)

                dr3 = rowt[:].rearrange("p (r j) -> p r j", j=NCH)
                for j in range(NCH):
                    nc.tensor.matmul(
                        psw[:], dr3[:, :, j],
                        x4[:, :, :, j].rearrange("p h c -> p (h c)"),
                        start=(g == 0 and j == 0),
                        stop=(g == g_seg - 1 and j == NCH - 1),
                        skip_group_check=True)

            # spill window into grid slice
            gsl_re = grid_re[:, bass.ts(s, WCOLS)]
            gsl_im = grid_im[:, bass.ts(s, WCOLS)]
            nc.vector.tensor_tensor(gsl_re, gsl_re, psw[:, 0:WCOLS],
                                    mybir.AluOpType.add)
            nc.vector.tensor_tensor(gsl_im, gsl_im, psw[:, WCOLS:2 * WCOLS],
                                    mybir.AluOpType.add)

        nc.sync.dma_start(out_re[:, :], grid_re[:])
        nc.sync.dma_start(out_im[:, :], grid_im[:])


def _bank_host(P_re, P_im, dF, dT, logit_gate):
    braw = np.zeros((66, 48), dtype=np.float32)
    braw[0:K_PAT, 0:9] = dF
    braw[0:K_PAT, 9:18] = dT
    braw[0:K_PAT, 18:27] = P_re
    braw[0:K_PAT, 27:36] = P_im
    braw[64, 0:9] = 1.0
    braw[65, 9:18] = 1.0
    glog = np.full((66, 48), 30.0, dtype=np.float32)
    glog[0:K_PAT, 18:27] = logit_gate
    glog[0:K_PAT, 27:36] = logit_gate
    return braw, glog


def kernel(tau, alpha, zeta_f, zeta_t, log_rho, theta, P_re, P_im, dF, dT,
           logit_gate):
    alpha = np.asarray(alpha, dtype=np.float32)
    zeta_f = np.asarray(zeta_f, dtype=np.float32)
    zeta_t = np.asarray(zeta_t, dtype=np.float32)
    log_rho = np.asarray(log_rho, dtype=np.float32)
    theta = np.asarray(theta, dtype=np.float32)
    P_re = np.asarray(P_re, dtype=np.float32)
    P_im = np.asarray(P_im, dtype=np.float32)
    dF = np.asarray(dF, dtype=np.float32)
    dT = np.asarray(dT, dtype=np.float32)
    logit_gate = np.asarray(logit_gate, dtype=np.float32)
    tau_f = float(np.asarray(tau))
    tau_neg = tau_f < 0.0

    alphas, vecs, g_seg = _host_shard(tau, alpha, zeta_f, zeta_t, log_rho,
                                      theta, dF, dT)
    nc = _build_program(g_seg, tau_neg)

    braw, glog = _bank_host(P_re, P_im, dF, dT, logit_gate)
    in_maps = []
    for c in range(NCORES):
        in_maps.append({
            "alpha_p": alphas[c],
            "vec_p": vecs[c],
            "bank_raw": braw, "gate_log": glog,
        })

    from concourse.bass_utils import run_bass_kernel_spmd
    trace = os.environ.get("CODEC_TRACE", "0") == "1"
    res = run_bass_kernel_spmd(nc, in_maps, list(range(NCORES)), trace=trace)
    global LAST_EXEC_NS, LAST_PROFILE
    LAST_EXEC_NS = res.exec_time_ns
    LAST_PROFILE = res.profile_json

    A = np.empty((F_DIM, N_DIM), dtype=np.complex64)
    for c in range(NCORES):
        r = res.results[c]
        A[c * ROWS:(c + 1) * ROWS] = r["out_re"] + 1j * r["out_im"]
    return A


if __name__ == "__main__":
    # smoke test with small M via the same code path
    rng = np.random.default_rng(0)
    M = 4096
    inputs = dict(
        tau=1,
        alpha=rng.standard_normal((M, K_PAT), dtype=np.float32),
        zeta_f=rng.random(M, dtype=np.float32) * np.float32(TWO_PI),
        zeta_t=rng.random(M, dtype=np.float32) * np.float32(TWO_PI),
        log_rho=rng.standard_normal(M, dtype=np.float32) * 0.1 - 1.0,
        theta=rng.random(M, dtype=np.float32) * np.float32(TWO_PI),
        P_re=rng.standard_normal((K_PAT, NCH), dtype=np.float32),
        P_im=rng.standard_normal((K_PAT, NCH), dtype=np.float32),
        dF=rng.standard_normal((K_PAT, NCH), dtype=np.float32) * 0.5,
        dT=rng.standard_normal((K_PAT, NCH), dtype=np.float32) * 0.5,
        logit_gate=rng.standard_normal((K_PAT, NCH), dtype=np.float32),
    )
    A = kernel(**inputs)
    print("out", A.shape, A.dtype, np.abs(A).sum())


# revision 43
# speedup vs baseline: 1.0560x; 1.0560x over previous
"""Trainium2 kernel for nn_CodecModel (scatter_memory).

Pipeline (8 NeuronCores, SPMD, full inputs in / full output out):
  Host (sharding only, no model math):
    - f_c/n_c centers from zeta_f/zeta_t (cheap elementwise, used only to BIN
      occurrences); each occurrence is routed to the (core=row-range-of-128,
      window=col-range-of-64) pairs its deposit footprint can touch
      (conservative bounds from global min/max of dF/dT). Duplicates are
      exact-split on device by construction (tents only match in-range rows,
      windows only cover their own columns).
  Device (per core, identical static program):
    - argmax over K=64 via row-max + is_equal one-hot
    - pattern gather via PE matmul with the transposed one-hot as stationary;
      f_c/n_c ride along as extra contraction rows so psum directly yields
      f_hat/n_hat [128,9] plus gathered P*sigmoid(gate)
    - bilinear scatter-add: for each 128-occurrence tile and channel j,
      stationary = row-tent [pt,128rows], moving = col-tent*val [pt,128
      (re|im)], accumulated over a whole column window in PSUM, spilled to an
      SBUF-resident [128,4096] re/im grid slice.
  Output: concatenate per-core row slices; complex64.
"""

import os
import sys

import numpy as np

sys.path.insert(0, "/opt/trn_rl_repo")

import concourse.bass as bass  # noqa: E402
import concourse.tile as tile  # noqa: E402
from concourse import mybir  # noqa: E402

F_DIM, N_DIM, K_PAT, NCH = 1024, 4096, 64, 9
TWO_PI = 2.0 * np.pi
NCORES = 8
ROWS = F_DIM // NCORES  # 128 rows per core
NWIN = 64  # column windows per core
WCOLS = N_DIM // NWIN  # 64 cols per window
FP32 = mybir.dt.float32
BF16 = mybir.dt.bfloat16
I32 = mybir.dt.int32

_DEBUG_SIM = os.environ.get("CODEC_KERNEL_SIM", "0") == "1"
LAST_EXEC_NS = None
LAST_PROFILE = None


def _host_shard(tau, alpha, zeta_f, zeta_t, log_rho, theta, dF, dT):
    """Bin occurrences into (core, window) segments. Returns per-core packed
    alpha and per-occurrence vectors plus the global segment size G_seg."""
    M = alpha.shape[0]
    f_c = (np.float32(F_DIM / TWO_PI) * zeta_f.astype(np.float32)).astype(np.float32)
    n_c = (np.float32(N_DIM / TWO_PI) * zeta_t.astype(np.float32)).astype(np.float32)

    f64c = f_c.astype(np.float64)
    n64c = n_c.astype(np.float64)
    dFmin, dFmax = float(dF.min()), float(dF.max())
    dTmin, dTmax = float(dT.min()), float(dT.max())
    # conservative deposit row/col spans (corner cells), +-1 slack
    row_lo = np.floor(f64c + dFmin).astype(np.int64) - 1
    row_hi = np.floor(f64c + dFmax).astype(np.int64) + 2
    col_lo = np.floor(n64c + dTmin).astype(np.int64) - 1
    col_hi = np.floor(n64c + dTmax).astype(np.int64) + 2
    np.clip(row_lo, 0, F_DIM - 1, out=row_lo)
    np.clip(row_hi, 0, F_DIM - 1, out=row_hi)
    np.clip(col_lo, 0, N_DIM - 1, out=col_lo)
    np.clip(col_hi, 0, N_DIM - 1, out=col_hi)
    c_lo, c_hi = row_lo // ROWS, row_hi // ROWS
    w_lo, w_hi = col_lo // WCOLS, col_hi // WCOLS

    occs, keys = [], []
    for dc in (0, 1):
        for dw in (0, 1):
            m = (c_lo + dc <= c_hi) & (w_lo + dw <= w_hi)
            idx = np.nonzero(m)[0]
            occs.append(idx)
            keys.append((c_lo[idx] + dc) * NWIN + (w_lo[idx] + dw))
    occs = np.concatenate(occs)
    keys = np.concatenate(keys).astype(np.int32)
    order = np.argsort(keys, kind="stable")
    occs, keys = occs[order], keys[order]

    counts = np.bincount(keys, minlength=NCORES * NWIN)
    g_seg = int(max(1, -(-counts.max() // 128)))
    if g_seg > 2:
        g_seg = -(-g_seg // 4) * 4  # round up for NEFF-cache stability
    cap = g_seg * 128
    sel = np.full((NCORES * NWIN, cap), -1, dtype=np.int64)
    starts = np.concatenate(([0], np.cumsum(counts)[:-1]))
    within = np.arange(len(occs)) - starts[keys]
    sel[keys, within] = occs
    sel = sel.reshape(NCORES, NWIN * cap)

    alphas, vecs = [], []
    win_of = np.repeat(np.arange(NWIN, dtype=np.float32) * WCOLS, cap)
    for c in range(NCORES):
        s = sel[c]
        valid = s >= 0
        sc = np.maximum(s, 0)
        a = alpha[sc]  # [L, 64]
        v = np.empty((s.shape[0], 4), dtype=np.float32)
        v[:, 0] = np.where(valid, f_c[sc] - np.float32(c * ROWS), np.float32(-512.0))
        v[:, 1] = np.where(valid, n_c[sc] - win_of, np.float32(0.0))
        v[:, 2] = np.where(valid, log_rho[sc], np.float32(0.0))
        v[:, 3] = np.where(valid, theta[sc], np.float32(0.0))
        alphas.append(np.ascontiguousarray(a))
        vecs.append(v)
    return alphas, vecs, g_seg


def _build_program(g_seg, tau_neg, chunk=8):
    """Build the SPMD Bass program for one core (identical across cores)."""
    from concourse import bacc
    nc = bacc.Bacc("TRN2", target_bir_lowering=False, debug=False)
    L = NWIN * g_seg * 128
    a_in = nc.dram_tensor("alpha_p", [L, K_PAT], FP32, kind="ExternalInput").ap()
    v_in = nc.dram_tensor("vec_p", [L, 4], FP32, kind="ExternalInput").ap()
    braw_in = nc.dram_tensor("bank_raw", [66, 48], FP32, kind="ExternalInput").ap()
    glog_in = nc.dram_tensor("gate_log", [66, 48], FP32, kind="ExternalInput").ap()
    out_re = nc.dram_tensor("out_re", [ROWS, N_DIM], FP32, kind="ExternalOutput").ap()
    out_im = nc.dram_tensor("out_im", [ROWS, N_DIM], FP32, kind="ExternalOutput").ap()

    ntiles = NWIN * g_seg
    assert ntiles % chunk == 0 or chunk == 1

    with tile.TileContext(nc) as tc:
        _emit(tc, nc, g_seg, tau_neg, chunk,
              a_in, v_in, braw_in, glog_in, out_re, out_im)
    nc.compile()
    return nc


def _emit(tc, nc, g_seg, tau_neg, chunk,
          a_in, v_in, braw_in, glog_in, out_re, out_im):
    from contextlib import ExitStack
    ctx = ExitStack()
    with ctx:
        cpool = ctx.enter_context(tc.tile_pool(name="const", bufs=1))
        gpool = ctx.enter_context(tc.tile_pool(name="grid", bufs=1))
        apool = ctx.enter_context(tc.tile_pool(name="alpha", bufs=3))
        vpool = ctx.enter_context(tc.tile_pool(name="vec", bufs=3))
        wpool = ctx.enter_context(tc.tile_pool(name="work", bufs=3))
        dpool = ctx.enter_context(tc.tile_pool(name="dep", bufs=3))
        spool = ctx.enter_context(tc.tile_pool(name="stat", bufs=3))
        pwin = ctx.enter_context(tc.tile_pool(name="pwin", bufs=2, space="PSUM"))
        pocc = ctx.enter_context(tc.tile_pool(name="pocc", bufs=2, space="PSUM"))
        ptr = ctx.enter_context(tc.tile_pool(name="ptr", bufs=2, space="PSUM"))

        # ---- constants ----
        iota_r_i = cpool.tile([128, 128], FP32)
        nc.gpsimd.iota(iota_r_i[:], pattern=[[1, 128]], base=0, channel_multiplier=0,
                       allow_small_or_imprecise_dtypes=True)
        iota_p_i = cpool.tile([128, 1], FP32)
        nc.gpsimd.iota(iota_p_i[:], pattern=[[0, 1]], base=0, channel_multiplier=1,
                       allow_small_or_imprecise_dtypes=True)
        ident = cpool.tile([128, 128], FP32)
        nc.vector.tensor_scalar(ident[:], iota_r_i[:], iota_p_i[:], None,
                                mybir.AluOpType.is_equal)
        # r-major row iota [p, r*9+j] = r, bf16; c-major col iota [p, c*9+j] = c
        iota_rmaj = cpool.tile([128, 128 * NCH], BF16)
        nc.gpsimd.iota(iota_rmaj[:], pattern=[[1, 128], [0, NCH]], base=0,
                       channel_multiplier=0, allow_small_or_imprecise_dtypes=True)
        iota_cmaj = cpool.tile([128, WCOLS * NCH], BF16)
        nc.gpsimd.iota(iota_cmaj[:], pattern=[[1, WCOLS], [0, NCH]], base=0,
                       channel_multiplier=0, allow_small_or_imprecise_dtypes=True)

        # ---- pattern bank: bank = bank_raw * sigmoid(gate_logits) ----
        braw = cpool.tile([66, 48], FP32)
        nc.sync.dma_start(braw[:], braw_in[:, :])
        glog = cpool.tile([66, 48], FP32)
        nc.sync.dma_start(glog[:], glog_in[:, :])
        gsig = cpool.tile([66, 48], FP32)
        nc.scalar.activation(gsig[:], glog[:], mybir.ActivationFunctionType.Sigmoid)
        braw_s = cpool.tile([66, 48], FP32)
        nc.scalar.copy(braw_s[:], braw[:])  # funnel deps through ACT clock
        bank = cpool.tile([66, 48], FP32)
        nc.vector.tensor_tensor(bank[:], braw_s[:], gsig[:], mybir.AluOpType.mult)

        negpi = cpool.tile([128, 1], FP32)
        nc.vector.memset(negpi[:], float(-np.pi))
        negone = cpool.tile([128, 1], FP32)
        nc.vector.memset(negone[:], -1.0)
        one = cpool.tile([128, 1], FP32)
        nc.vector.memset(one[:], 1.0)

        # ---- grid ----
        grid_re = gpool.tile([128, N_DIM], FP32)
        grid_im = gpool.tile([128, N_DIM], FP32)
        nc.vector.memset(grid_re[:], 0.0)
        nc.vector.memset(grid_im[:], 0.0)

        red_op = mybir.AluOpType.min if tau_neg else mybir.AluOpType.max

        # ---- amplitude prologue: amp for ALL tiles, 3 ACT table loads total
        G = NWIN * g_seg
        vall = gpool.tile([128, G * 4], FP32)
        nc.sync.dma_start(
            vall[:].rearrange("p (c k) -> p c k", k=4),
            v_in[:, :].rearrange("(c p) k -> p c k", p=128))
        v3 = vall[:].rearrange("p (c k) -> p k c", k=4)
        rho_all = gpool.tile([128, G], FP32)
        nc.scalar.activation(rho_all[:], v3[:, 2], mybir.ActivationFunctionType.Exp)
        thr = gpool.tile([128, G], FP32, tag="thr")
        gg = gpool.tile([128, G], FP32, tag="gg")
        cth_all = gpool.tile([128, G], FP32)
        sth_all = gpool.tile([128, G], FP32)
        for dst, shift in ((cth_all, 1.5 * np.pi), (sth_all, np.pi)):
            nc.vector.tensor_scalar(thr[:], v3[:, 3], float(shift), None,
                                    mybir.AluOpType.add)
            nc.vector.tensor_scalar(gg[:], thr[:], float(TWO_PI), None,
                                    mybir.AluOpType.is_ge)
            nc.vector.scalar_tensor_tensor(thr[:], gg[:], float(-TWO_PI), thr[:],
                                           mybir.AluOpType.mult,
                                           mybir.AluOpType.add)
            nc.scalar.activation(dst[:], thr[:], mybir.ActivationFunctionType.Sin,
                                 bias=negpi[:])
        are_all = gpool.tile([128, G], FP32)
        nc.vector.tensor_tensor(are_all[:], rho_all[:], cth_all[:],
                                mybir.AluOpType.mult)
        aim_all = gpool.tile([128, G], FP32)
        nc.vector.tensor_tensor(aim_all[:], rho_all[:], sth_all[:],
                                mybir.AluOpType.mult)
        aimn_all = gpool.tile([128, G], FP32)
        nc.vector.tensor_scalar(aimn_all[:], aim_all[:], -1.0, None,
                                mybir.AluOpType.mult)

        seg_rows = g_seg * 128
        with tc.For_i(0, NWIN, 1,
                      hint_engines=(mybir.EngineType.DVE,
                                    mybir.EngineType.Activation,
                                    mybir.EngineType.PE,
                                    mybir.EngineType.SP)) as s:
            a_chunk = apool.tile([128, g_seg * K_PAT], FP32, tag="ach")
            src = a_in[bass.ds(s * seg_rows, seg_rows), :].rearrange(
                "(c p) k -> p c k", p=128)
            nc.sync.dma_start(
                a_chunk[:].rearrange("p (c k) -> p c k", k=K_PAT), src)

            psw = pwin.tile([128, 128], FP32)
            base4 = s * (g_seg * 4)
            base1 = s * g_seg
            for g in range(g_seg):
                ci = g
                at = a_chunk[:, ci * K_PAT:(ci + 1) * K_PAT]
                fcol = vall[:, bass.ds(base4 + ci * 4 + 0, 1)]
                ncol = vall[:, bass.ds(base4 + ci * 4 + 1, 1)]
                are = are_all[:, bass.ds(base1 + ci, 1)]
                aim = aim_all[:, bass.ds(base1 + ci, 1)]
                aimn = aimn_all[:, bass.ds(base1 + ci, 1)]

                # argmax one-hot (+centers) and transpose
                mx = wpool.tile([128, 1], FP32, tag="mx")
                nc.vector.tensor_reduce(mx[:], at, mybir.AxisListType.X, red_op)
                trin = wpool.tile([128, 66], FP32, tag="trin")
                nc.vector.tensor_scalar(trin[:, 0:64], at, mx[:], None,
                                        mybir.AluOpType.is_equal)
                nc.vector.tensor_copy(trin[:, 64:65], fcol)
                nc.vector.tensor_copy(trin[:, 65:66], ncol)
                pst = ptr.tile([66, 128], FP32, tag="pst")
                nc.tensor.transpose(pst[:], trin[:], ident[:])
                statg = spool.tile([66, 128], FP32, tag="statg")
                nc.scalar.copy(statg[:], pst[:])

                # gather matmul -> [128 occ, 48]: fhat|nhat|PGre|PGim
                pso = pocc.tile([128, 48], FP32, tag="pso")
                nc.tensor.matmul(pso[:], statg[:], bank[:], start=True, stop=True)
                occ = wpool.tile([128, 48], FP32, tag="occ")
                nc.vector.tensor_copy(occ[:], pso[:])

                # staging: [0:18]=f0|t0  [18:36]=wf|wt  [36:45]=vre  [45:54]=vim
                sf = wpool.tile([128, 54], FP32, tag="sf")
                MAGIC = 8388608.0  # 2**23: x+MAGIC-MAGIC rounds to nearest int
                rnd = wpool.tile([128, 18], FP32, tag="rnd")
                nc.vector.tensor_scalar(rnd[:], occ[:, 0:18], MAGIC, MAGIC,
                                        mybir.AluOpType.add,
                                        mybir.AluOpType.subtract)
                ggt = wpool.tile([128, 18], FP32, tag="ggt")
                nc.vector.tensor_tensor(ggt[:], rnd[:], occ[:, 0:18],
                                        mybir.AluOpType.is_gt)
                nc.vector.tensor_tensor(sf[:, 0:18], rnd[:], ggt[:],
                                        mybir.AluOpType.subtract)
                nc.vector.tensor_tensor(sf[:, 18:36], occ[:, 0:18], sf[:, 0:18],
                                        mybir.AluOpType.subtract)
                t1 = wpool.tile([128, 9], FP32, tag="t1")
                nc.vector.tensor_scalar(t1[:], occ[:, 18:27], are, None,
                                        mybir.AluOpType.mult)
                nc.vector.scalar_tensor_tensor(sf[:, 36:45], occ[:, 27:36], aimn,
                                               t1[:], mybir.AluOpType.mult,
                                               mybir.AluOpType.add)
                t2 = wpool.tile([128, 9], FP32, tag="t2")
                nc.vector.tensor_scalar(t2[:], occ[:, 18:27], aim, None,
                                        mybir.AluOpType.mult)
                nc.vector.scalar_tensor_tensor(sf[:, 45:54], occ[:, 27:36], are,
                                               t2[:], mybir.AluOpType.mult,
                                               mybir.AluOpType.add)
                stg = wpool.tile([128, 54], BF16, tag="stg")
                nc.vector.tensor_copy(stg[:], sf[:])

                # row tents [p, r*9+j] bf16
                drow = dpool.tile([128, 128 * NCH], BF16, tag="drow")
                d3 = drow[:].rearrange("p (r j) -> p r j", j=NCH)
                nc.vector.tensor_tensor(
                    d3, iota_rmaj[:].rearrange("p (r j) -> p r j", j=NCH),
                    stg[:, 0:9].unsqueeze(1).broadcast_to((128, 128, NCH)),
                    mybir.AluOpType.subtract)
                nc.vector.tensor_tensor(
                    d3, d3, stg[:, 18:27].unsqueeze(1).broadcast_to((128, 128, NCH)),
                    mybir.AluOpType.subtract)
                rowt = dpool.tile([128, 128 * NCH], BF16, tag="rowt")
                nc.scalar.activation(rowt[:], drow[:],
                                     mybir.ActivationFunctionType.Abs)
                nc.scalar.activation(rowt[:], rowt[:],
                                     mybir.ActivationFunctionType.Relu,
                                     scale=negone[:], bias=one[:])

                # col tents * val -> moving [p, (re|im) c*9+j] bf16
                dcol = dpool.tile([128, WCOLS * NCH], BF16, tag="dcol")
                c3 = dcol[:].rearrange("p (c j) -> p c j", j=NCH)
                nc.vector.tensor_tensor(
                    c3, iota_cmaj[:].rearrange("p (c j) -> p c j", j=NCH),
                    stg[:, 9:18].unsqueeze(1).broadcast_to((128, WCOLS, NCH)),
                    mybir.AluOpType.subtract)
                nc.vector.tensor_tensor(
                    c3, c3, stg[:, 27:36].unsqueeze(1).broadcast_to((128, WCOLS, NCH)),
                    mybir.AluOpType.subtract)
                ca = dpool.tile([128, WCOLS * NCH], BF16, tag="ca")
                nc.vector.tensor_scalar(ca[:], dcol[:], -1.0, 1.0,
                                        mybir.AluOpType.mult,
                                        mybir.AluOpType.add)
                nc.vector.scalar_tensor_tensor(dcol[:], dcol[:], 1.0, ca[:],
                                               mybir.AluOpType.add,
                                               mybir.AluOpType.min)
                nc.vector.tensor_scalar(dcol[:], dcol[:], 0.0, None,
                                        mybir.AluOpType.max)
                xrei = dpool.tile([128, 2 * WCOLS * NCH], BF16, tag="xrei")
                x4 = xrei[:].rearrange("p (h c j) -> p h c j", h=2, j=NCH)
                nc.vector.tensor_tensor(
                    x4[:, 0], c3,
                    stg[:, 36:45].unsqueeze(1).broadcast_to((128, WCOLS, NCH)),
                    mybir.AluOpType.mult)
                nc.vector.tensor_tensor(
                    x4[:, 1], c3,
                    stg[:, 45:54].unsqueeze(1).broadcast_to((128, WCOLS, NCH)),
                    mybir.AluOpType.mult)

                dr3 = rowt[:].rearrange("p (r j) -> p r j", j=NCH)
                for j in range(NCH):
                    nc.tensor.matmul(
                        psw[:], dr3[:, :, j],
                        x4[:, :, :, j].rearrange("p h c -> p (h c)"),
                        start=(g == 0 and j == 0),
                        stop=(g == g_seg - 1 and j == NCH - 1),
                        skip_group_check=True)

            # spill window into grid slice
            gsl_re = grid_re[:, bass.ts(s, WCOLS)]
            gsl_im = grid_im[:, bass.ts(s, WCOLS)]
            nc.vector.tensor_tensor(gsl_re, gsl_re, psw[:, 0:WCOLS],
                                    mybir.AluOpType.add)
            nc.vector.tensor_tensor(gsl_im, gsl_im, psw[:, WCOLS:2 * WCOLS],
                                    mybir.AluOpType.add)

        nc.sync.dma_start(out_re[:, :], grid_re[:])
        nc.sync.dma_start(out_im[:, :], grid_im[:])


def _bank_host(P_re, P_im, dF, dT, logit_gate):
    braw = np.zeros((66, 48), dtype=np.float32)
    braw[0:K_PAT, 0:9] = dF
    braw[0:K_PAT, 9:18] = dT
    braw[0:K_PAT, 18:27] = P_re
    braw[0:K_PAT, 27:36] = P_im
    braw[64, 0:9] = 1.0
    braw[65, 9:18] = 1.0
    glog = np.full((66, 48), 30.0, dtype=np.float32)
    glog[0:K_PAT, 18:27] = logit_gate
    glog[0:K_PAT, 27:36] = logit_gate
    return braw, glog


def kernel(tau, alpha, zeta_f, zeta_t, log_rho, theta, P_re, P_im, dF, dT,
           logit_gate):
    alpha = np.asarray(alpha, dtype=np.float32)
    zeta_f = np.asarray(zeta_f, dtype=np.float32)
    zeta_t = np.asarray(zeta_t, dtype=np.float32)
    log_rho = np.asarray(log_rho, dtype=np.float32)
    theta = np.asarray(theta, dtype=np.float32)
    P_re = np.asarray(P_re, dtype=np.float32)
    P_im = np.asarray(P_im, dtype=np.float32)
    dF = np.asarray(dF, dtype=np.float32)
    dT = np.asarray(dT, dtype=np.float32)
    logit_gate = np.asarray(logit_gate, dtype=np.float32)
    tau_f = float(np.asarray(tau))
    tau_neg = tau_f < 0.0

    alphas, vecs, g_seg = _host_shard(tau, alpha, zeta_f, zeta_t, log_rho,
                                      theta, dF, dT)
    nc = _build_program(g_seg, tau_neg)

    braw, glog = _bank_host(P_re, P_im, dF, dT, logit_gate)
    in_maps = []
    for c in range(NCORES):
        in_maps.append({
            "alpha_p": alphas[c],
            "vec_p": vecs[c],
            "bank_raw": braw, "gate_log": glog,
        })

    from concourse.bass_utils import run_bass_kernel_spmd
    trace = os.environ.get("CODEC_TRACE", "0") == "1"
    res = run_bass_kernel_spmd(nc, in_maps, list(range(NCORES)), trace=trace)
    global LAST_EXEC_NS, LAST_PROFILE
    LAST_EXEC_NS = res.exec_time_ns
    LAST_PROFILE = res.profile_json

    A = np.empty((F_DIM, N_DIM), dtype=np.complex64)
    for c in range(NCORES):
        r = res.results[c]
        A[c * ROWS:(c + 1) * ROWS] = r["out_re"] + 1j * r["out_im"]
    return A


if __name__ == "__main__":
    # smoke test with small M via the same code path
    rng = np.random.default_rng(0)
    M = 4096
    inputs = dict(
        tau=1,
        alpha=rng.standard_normal((M, K_PAT), dtype=np.float32),
        zeta_f=rng.random(M, dtype=np.float32) * np.float32(TWO_PI),
        zeta_t=rng.random(M, dtype=np.float32) * np.float32(TWO_PI),
        log_rho=rng.standard_normal(M, dtype=np.float32) * 0.1 - 1.0,
        theta=rng.random(M, dtype=np.float32) * np.float32(TWO_PI),
        P_re=rng.standard_normal((K_PAT, NCH), dtype=np.float32),
        P_im=rng.standard_normal((K_PAT, NCH), dtype=np.float32),
        dF=rng.standard_normal((K_PAT, NCH), dtype=np.float32) * 0.5,
        dT=rng.standard_normal((K_PAT, NCH), dtype=np.float32) * 0.5,
        logit_gate=rng.standard_normal((K_PAT, NCH), dtype=np.float32),
    )
    A = kernel(**inputs)
    print("out", A.shape, A.dtype, np.abs(A).sum())
